# revision 35
# baseline (speedup 1.0000x reference)
"""BitLinear forward on 8 Trainium2 NeuronCores.

Computes y = x @ (unpack_bits(bp).reshape(OUT, IN) * scale).T for
x[64, 4096] fp32, bp[OUT*IN/8] int32 (8 sign bits per int, MSB-first),
scale[OUT, 1] fp32, OUT=11008, IN=4096.

Strategy (column-parallel / output-feature sharded, no collectives):
  * Each core owns 1408 = 11*128 padded output rows.
  * Host packs the weight bytes as uint16 pairs of ADJACENT OUTPUTS:
    bpt2[g, m] = byte(o=2m) | byte(o=2m+1) << 8, shape [512, 704].
  * Device unpack: ONE dual-op bitwise tensor_scalar per bit plane,
    (v << (6-p)) & 0x4040 (p=7: v >> 1), which lands each byte's bit p
    at fp8_e4m3 position 6 of its own byte -> the u16 result, bitcast
    to fp8, is the {0.0, 2.0}-valued plane for outputs (2m, 2m+1) in
    natural order. 4x-mode DVE, ~0.9us per [128, 2816] op.
  * PE: mixed-dtype matmuls psum[t, o] += xt_chunk.T @ plane_fp8 with
    bf16 stationary weights (full x precision) and fp8 moving planes.
    Column tiling: c-even chunks stream on array cols 0:64 (psum rows
    0:64), c-odd on 64:128 -- two concurrent, balanced streams. A
    burst of dummy matmuls right after the preamble warms the PE HAM
    clock gate so the real stream runs at 2.4 GHz.
  * DMA: bpt g-chunks split across the sync + scalar rings, xt in four
    chunks on the gpsimd ring, ordered by first use; lo-half (c0, c1)
    planes run before the hi half so late c2/c3 arrivals stay hidden.
  * No epilogue math on device: psum holds 2*sum(b*x) partials; DVE +
    ACT copy psum -> SBUF fp16 per og chunk, DMA out on both hw rings.
    Host computes y = (even_rows + odd_rows - sum(x)) * scale and
    re-assembles.
"""

import numpy as np
import ml_dtypes

OUT, IN, TOKENS = 11008, 4096, 64
NCORES = 8
P = 128
G = IN // 8              # 512 in-feature groups (bytes per output row)
OPC = 1408               # padded output rows per core (11 * 128)
OPC2 = OPC // 2          # 704 u16 output-pairs per core
OUT_PAD = NCORES * OPC   # 11264
OG_SIZES = [512, 512, 384]
OG_STARTS = [0, 512, 1024]

_CACHE = {}


def _build_bass():
    """Build + compile the per-core Bass kernel (identical on all cores)."""
    from contextlib import ExitStack

    import concourse.mybir as mybir
    import concourse.tile as tile
    from concourse import bacc

    nc = bacc.Bacc("TRN2", target_bir_lowering=False, debug=False)

    bpt = nc.dram_tensor("bpt", (G, OPC2), mybir.dt.uint16, kind="ExternalInput")
    xt = nc.dram_tensor("xt", (P, 32 * TOKENS), mybir.dt.bfloat16, kind="ExternalInput")
    yt = nc.dram_tensor("yt", (P, OPC), mybir.dt.float16, kind="ExternalOutput")

    with tile.TileContext(nc) as tc, ExitStack() as ctx:
        consts = ctx.enter_context(tc.tile_pool(name="consts", bufs=1))
        plane_pool = ctx.enter_context(tc.tile_pool(name="planes", bufs=8))
        hi_pool = ctx.enter_context(tc.tile_pool(name="hiplanes", bufs=8))
        out_pool = ctx.enter_context(tc.tile_pool(name="outs", bufs=1))
        psum_y = ctx.enter_context(tc.tile_pool(name="psum_y", bufs=1, space="PSUM"))

        bpt_all = consts.tile([P, 4 * OPC2], mybir.dt.uint16, name="bpt_all")
        xt_s = consts.tile([P, 32 * TOKENS], mybir.dt.bfloat16, name="xt_s")

        # --- inputs: fat contiguous transfers, 3 rings in parallel ---
        # sync: c0 then c2; scalar: c1 then c3; gpsimd: xt in 4 chunks
        # ordered by first use (m-chunk p*4+c, plane-major).
        for c in range(4):
            ring = nc.sync if c % 2 == 0 else nc.scalar
            ring.dma_start(bpt_all[:, c * OPC2:(c + 1) * OPC2],
                           bpt[c * P:(c + 1) * P, :])
        for a, b in ((0, 8), (8, 16), (16, 24), (24, 32)):
            nc.gpsimd.dma_start(xt_s[:, a * TOKENS:b * TOKENS],
                                xt[:, a * TOKENS:b * TOKENS])

        out_s = out_pool.tile([P, OPC], mybir.dt.float16, name="out_s")

        # 3 og psum tiles; rows 0:64 accumulate the c-even stream (PE
        # array cols 0:64), rows 64:128 the c-odd stream.
        pts = [
            psum_y.tile([P, w], mybir.dt.float32, name=f"psum_{og}")
            for og, w in enumerate(OG_SIZES)
        ]

        # --- PE warm-up: ~4us of dummy matmuls so HAM reaches 8/8 before
        # the real stream starts (PE is otherwise idle during input DMA).
        warm_src = consts.tile([P, 512], mybir.dt.bfloat16, name="warm_src")
        warm_ps = psum_y.tile([P, 512], mybir.dt.float32, name="warm_ps")
        nc.vector.memset(warm_src[:], 0.0)
        for i in range(6):
            nc.tensor.matmul(warm_ps[0:TOKENS, :], warm_src[:, :TOKENS],
                             warm_src[:], start=True, stop=True,
                             tile_position=(0, 0))

        def _ts_extract(dst_ap, src_ap, p):
            if p <= 6:
                nc.vector.tensor_scalar(
                    dst_ap, src_ap, 6 - p, 0x4040,
                    mybir.AluOpType.logical_shift_left,
                    mybir.AluOpType.bitwise_and)
            else:
                nc.vector.tensor_scalar(
                    dst_ap, src_ap, 1, 0x4040,
                    mybir.AluOpType.logical_shift_right,
                    mybir.AluOpType.bitwise_and)

        def extract(p, half, dst):
            """Bit-plane p of c-chunks (2h, 2h+1) -> dst u16 [P, 1408].

            dst holds 0x40*bit per byte; bitcast to fp8 = {0,2} plane.
            """
            _ts_extract(dst[:],
                        bpt_all[:, half * 2 * OPC2:(half + 1) * 2 * OPC2], p)

        def plane_mm(pl8, p, c, s0, w, og, start=False, stop=False):
            """pl8: fp8 AP [P, 2*OPC] covering chunks (c&~1, c|1).

            (s0, w): column slice within the 1408-wide og space.
            """
            m = p * 4 + c
            half = c % 2
            base = half * TOKENS
            nc.tensor.matmul(
                pts[og][base:base + TOKENS, s0 - OG_STARTS[og]:
                        s0 - OG_STARTS[og] + w],
                xt_s[:, m * TOKENS:(m + 1) * TOKENS],
                pl8[:, half * OPC + s0:half * OPC + s0 + w],
                start=start, stop=stop,
                tile_position=(0, base),
            )

        # --- unpack + matmul rounds ---
        # Lo phase (chunks c0, c1), plane-major: the c2/c3 DMAs land
        # later and psum accumulation order is free, so all lo planes
        # run first. Both matmuls of a pair hit different PE column
        # groups and stream concurrently.
        for p in range(8):
            u = plane_pool.tile([P, 2 * OPC2], mybir.dt.uint16, name="ulo")
            extract(p, 0, u)
            pl8 = u[:].bitcast(mybir.dt.float8e4)
            for og in range(3):
                plane_mm(pl8, p, 0, OG_STARTS[og], OG_SIZES[og], og,
                         start=(p == 0))
                plane_mm(pl8, p, 1, OG_STARTS[og], OG_SIZES[og], og,
                         start=(p == 0))

        # Hi phase (chunks c2, c3), OG-MAJOR: each og finishes all eight
        # planes as a block, so og0/og1's copy+DMA chains run fully
        # hidden under the remaining blocks' matmuls and only og2's
        # short (384-wide) chain trails the last matmul. All eight hi
        # planes are extracted up front (they stay live in hi_pool).
        hi_planes = []
        for p in range(8):
            u = hi_pool.tile([P, 2 * OPC2], mybir.dt.uint16, name="uhi")
            extract(p, 1, u)
            hi_planes.append(u[:].bitcast(mybir.dt.float8e4))

        for og in range(3):
            s0, w = OG_STARTS[og], OG_SIZES[og]
            for p in range(8):
                plane_mm(hi_planes[p], p, 2, s0, w, og, stop=(p == 7))
                plane_mm(hi_planes[p], p, 3, s0, w, og, stop=(p == 7))
            dst = out_s[:, s0:s0 + w]
            if og == 0:
                nc.vector.tensor_copy(dst, pts[og][:, :])
                nc.sync.dma_start(yt[:, s0:s0 + w], dst)
            elif og == 1:
                nc.scalar.copy(dst, pts[og][:, :])
                nc.scalar.dma_start(yt[:, s0:s0 + w], dst)
            else:
                # last chunk trails the final matmul: run two half
                # chains fully in parallel (DVE+ACT copies, sync+scalar
                # DMA issues -- all four are idle by now)
                h = w // 2
                nc.vector.tensor_copy(dst[:, :h], pts[og][:, :h])
                nc.scalar.copy(dst[:, h:], pts[og][:, h:])
                nc.sync.dma_start(yt[:, s0:s0 + h], dst[:, :h])
                nc.scalar.dma_start(yt[:, s0 + h:s0 + w], dst[:, h:])

    nc.compile()
    return nc


def _prep_inputs(x, bp, scale):
    """Host-side re-layout of the full inputs into 8 per-core input maps."""
    x = np.asarray(x, dtype=np.float32)
    bp = np.asarray(bp)

    # byte matrix [G, OUT_PAD]: byte of (output o, group g)
    bpm = np.zeros((G, OUT_PAD), dtype=np.uint8)
    bpm[:, :OUT] = bp.astype(np.uint8).reshape(OUT, G).T

    # xt chunks m = p*4 + c: lhsT[r, t] = x[t, 8*(c*128+r) + (7-p)]
    xT = x.T.astype(np.float32)                     # [IN, T]
    xg = xT.reshape(G, 8, TOKENS)                   # [g, i, t]; i = 7-p
    xpc = xg.reshape(4, P, 8, TOKENS)               # [c, r, i, t]
    xpc = xpc.transpose(2, 0, 1, 3)[::-1]           # [p, c, r, t]
    xt_dev = np.ascontiguousarray(
        xpc.transpose(2, 0, 1, 3).reshape(P, 32 * TOKENS)
    ).astype(ml_dtypes.bfloat16)

    in_maps = []
    for cid in range(NCORES):
        sl = slice(cid * OPC, (cid + 1) * OPC)
        bslice = np.ascontiguousarray(bpm[:, sl])   # [G, OPC] u8
        bpt2 = bslice.view("<u2")                   # [G, OPC2] u16 pairs
        in_maps.append({"bpt": bpt2, "xt": xt_dev})
    return in_maps


def _assemble(results, x, scale):
    """per-core yt [128, 1408] fp16 -> full [64, OUT] fp32."""
    x64 = np.asarray(x, dtype=np.float64)
    sumx = x64.sum(axis=1).astype(np.float32)       # [T]
    scale = np.asarray(scale, dtype=np.float32).reshape(-1)  # [OUT]

    full = np.empty((TOKENS, OUT), dtype=np.float32)
    for cid in range(NCORES):
        a = np.asarray(results[cid]["yt"], dtype=np.float32)  # [128, 1408]
        ydev = a[:TOKENS] + a[TOKENS:]              # [T, OPC]
        o0 = cid * OPC
        n = min(OPC, OUT - o0)
        if n <= 0:
            continue
        full[:, o0:o0 + n] = ydev[:, :n] - sumx[:, None]
    full *= scale[None, :]
    return full


def kernel(x, bp, scale, _trace=False):
    from concourse import bass_utils

    if "nc" not in _CACHE:
        _CACHE["nc"] = _build_bass()
    nc = _CACHE["nc"]

    in_maps = _prep_inputs(x, bp, scale)
    res = bass_utils.run_bass_kernel_spmd(
        nc, in_maps, core_ids=list(range(NCORES)), trace=_trace,
    )
    _CACHE["last_result"] = res
    return _assemble(res.results, x, scale)
